# revision 1
# baseline (speedup 1.0000x reference)
"""ConceptCLIP loss kernel for 8x Trainium2 NeuronCores (Bass/Tile).

Strategy (data-parallel over the image batch axis m):
  - Each core owns 16 of the 128 images: its patch shard (16,196,768) plus the
    full concept/text features (small) are shipped to every core.
  - Concepts are host-packed: only the w < counts[v] concepts take part
    (masked-out concepts contribute 0 to the loss), cutting ~half the FLOPs.
  - Concept L2 normalization is deferred: max_n(c_raw . p_norm) = ||c|| *
    max_n(c_norm . p_norm), so the 1/||c|| (computed on device) is applied to
    the max-pooled values instead of the big operand.
  - Device pipeline: normalize patches (ACT square+accum -> sqrt -> DVE recip
    -> scale) -> PE transpose to (d, n) layout -> big bf16 matmul
    A[p, m*n] accumulated over 6 K-chunks into PSUM (4 concurrent accumulation
    chains in 4 distinct PSUM banks, so each LDWEIGHTS feeds 4 matmuls) ->
    DVE reduce_max over patches -> fp32 matmul with a host-built gather matrix
    G (mask/counts) -> logits -> softplus loss elements. Host sums the
    per-element losses. Patch prep is interleaved with the main-loop image
    blocks so the PE never waits long for prepped images.
"""

import math
import os
import sys

for _p in ("/opt/trn_rl_repo", "/root/.axon_site/_ro/trn_rl_repo"):
    if os.path.isdir(_p) and _p not in sys.path:
        sys.path.insert(0, _p)

import ml_dtypes
import numpy as np

import concourse.tile as tile
from concourse import bacc, mybir
from concourse.bass_utils import run_bass_kernel_spmd

BF16 = ml_dtypes.bfloat16

N_CORES = 8
B, NPATCH, D, W = 128, 196, 768, 32
M_PER = B // N_CORES  # 16 images per core
KC = D // 128         # 6 contraction chunks

F32 = mybir.dt.float32
BF = mybir.dt.bfloat16
AX = mybir.AxisListType
AF = mybir.ActivationFunctionType

_cache = {}


def _build(C, t, bias):
    """Build + compile the per-core Bass program. C = number of 128-row packed
    concept chunks; t/bias are compile-time scalar constants."""
    P = C * 128
    nc = bacc.Bacc("TRN2", target_bir_lowering=False, debug=False,
                   num_devices=N_CORES)

    d_patches = nc.dram_tensor("patches", (M_PER, NPATCH, D), BF, kind="ExternalInput")
    d_cT = nc.dram_tensor("cT", (KC, 128, P), BF, kind="ExternalInput")
    d_cnat = nc.dram_tensor("cnat", (P, D), BF, kind="ExternalInput")
    d_GT = nc.dram_tensor("GT", (C, 128, B), F32, kind="ExternalInput")
    d_img = nc.dram_tensor("img", (M_PER, D), BF, kind="ExternalInput")
    d_txt = nc.dram_tensor("txt", (B, D), BF, kind="ExternalInput")
    d_sign = nc.dram_tensor("signneg", (B, M_PER), F32, kind="ExternalInput")
    d_ident = nc.dram_tensor("ident", (128, 128), BF, kind="ExternalInput")
    d_rc = nc.dram_tensor("rc_el", (B, M_PER), F32, kind="ExternalOutput")
    d_it = nc.dram_tensor("it_el", (B, M_PER), F32, kind="ExternalOutput")

    with tile.TileContext(nc) as tc:
        with (
            tc.tile_pool(name="consts", bufs=1) as consts,
            tc.tile_pool(name="work", bufs=3) as work,
            tc.tile_pool(name="small", bufs=4) as small,
            tc.tile_pool(name="psum", bufs=2, space="PSUM") as psum,
        ):
            sign = consts.tile([B, M_PER], F32, tag="sign")
            nc.sync.dma_start(out=sign[:], in_=d_sign.ap())
            ident = consts.tile([128, 128], BF, tag="ident")
            nc.sync.dma_start(out=ident[:], in_=d_ident.ap())
            warm = small.tile([1, 1], F32, tag="warm")
            nc.vector.memset(warm[:], 1.0)
            nc.scalar.activation(out=warm[:], in_=warm[:], func=AF.Square)
            txtT = consts.tile([128, KC, 128], BF, tag="txtT")
            imgT = consts.tile([128, KC, M_PER], BF, tag="imgT")
            rhs = [consts.tile([128, KC, NPATCH], BF, tag=f"rhs{m}", name=f"rhs{m}")
                   for m in range(M_PER)]
            maxcol = consts.tile([128, C, M_PER], F32, tag="maxcol")
            rnorm = consts.tile([128, C], F32, tag="rnorm")
            yit = consts.tile([B, M_PER], F32, tag="yit")

            def rownorm_recip(src_ap, nrows, rinv_ap):
                # rinv = 1 / ||row||_2 per partition (ACT square+accum path)
                scr = work.tile([128, D], BF, tag="scr", bufs=3)
                ssq = small.tile([128, 1], F32, tag="ssq", bufs=8)
                nc.scalar.activation(out=scr[:nrows], in_=src_ap,
                                     func=AF.Square, accum_out=ssq[:nrows])
                nc.scalar.sqrt(ssq[:nrows], ssq[:nrows])
                nc.vector.reciprocal(rinv_ap, ssq[:nrows])

            def norm_transpose(src_tile, nrows, dst, col0, copy_eng):
                # normalize rows of (nrows, 768) tile, PE-transpose each
                # 128-col chunk, copy PSUM->SBUF into dst[:, k, col0:col0+nrows]
                rinv = small.tile([128, 1], F32, tag="rinv", bufs=12)
                rownorm_recip(src_tile[:nrows], nrows, rinv[:nrows])
                nrm = work.tile([128, D], BF, tag="nrm", bufs=24)
                nc.vector.tensor_scalar_mul(nrm[:nrows], src_tile[:nrows],
                                            rinv[:nrows])
                ps = psum.tile([128, 1024], BF, tag="ps", name="ps_t")
                for k in range(KC):
                    nc.tensor.transpose(ps[:, k * nrows:(k + 1) * nrows],
                                        nrm[:nrows, k * 128:(k + 1) * 128],
                                        ident[:nrows, :nrows])
                src_view = ps[:, 0:KC * nrows].rearrange("p (k n) -> p k n", k=KC)
                copy_eng(out=dst[:, :, col0:col0 + nrows], in_=src_view)

            # patch prep: sumsq of the 128-row block on DVE (tensor_tensor_
            # reduce), of the 68-row block on ACT (square+accum); sqrt/recip
            # batched per image; copies on ACT; transposes on PE.
            PBLOCKS = ((0, 128), (128, NPATCH - 128))

            def prep_image(m):
                ssq2 = small.tile([128, 2], F32, tag="ssq2", bufs=8)
                rinv2 = small.tile([128, 2], F32, tag="rinv2", bufs=8)
                nats = []
                for b, (r0, nrows) in enumerate(PBLOCKS):
                    natp = work.tile([128, D], BF, tag="nat", bufs=12)
                    nc.sync.dma_start(out=natp[:nrows],
                                      in_=d_patches.ap()[m, r0:r0 + nrows, :])
                    nats.append(natp)
                    scr = work.tile([128, D], BF, tag="scr", bufs=3)
                    nc.scalar.activation(out=scr[:nrows], in_=natp[:nrows],
                                         func=AF.Square,
                                         accum_out=ssq2[:nrows, b:b + 1])
                for b, (r0, nrows) in enumerate(PBLOCKS):
                    nc.scalar.sqrt(ssq2[:nrows, b:b + 1], ssq2[:nrows, b:b + 1])
                    nc.vector.reciprocal(rinv2[:nrows, b:b + 1],
                                         ssq2[:nrows, b:b + 1])
                for b, (r0, nrows) in enumerate(PBLOCKS):
                    nrm = work.tile([128, D], BF, tag="nrm", bufs=24)
                    nc.vector.tensor_scalar_mul(nrm[:nrows], nats[b][:nrows],
                                                rinv2[:nrows, b:b + 1])
                    ps = psum.tile([128, 1024], BF, tag="ps", name="ps_t")
                    for k in range(KC):
                        nc.tensor.transpose(ps[:, k * nrows:(k + 1) * nrows],
                                            nrm[:nrows, k * 128:(k + 1) * 128],
                                            ident[:nrows, :nrows])
                    src_view = ps[:, 0:KC * nrows].rearrange(
                        "p (k n) -> p k n", k=KC)
                    eng = nc.vector.tensor_copy if m % 2 == 0 else nc.scalar.copy
                    eng(out=rhs[m][:, :, r0:r0 + nrows], in_=src_view)

            for m in range(4):
                prep_image(m)

            cT = []
            for k in range(KC):
                tk = consts.tile([128, P], BF, tag=f"cT{k}", name=f"cT{k}")
                nc.sync.dma_start(out=tk[:], in_=d_cT.ap()[k])
                cT.append(tk)
            for m in range(4, 8):
                prep_image(m)

            def main_pt(pt, preps=()):
                # A[concept_chunk, image, patch] -> max over patches. k-outer
                # with 4 concurrent accumulation chains in 4 distinct PSUM
                # banks so each LDWEIGHTS is reused by 4 matmuls.
                preps = dict(preps)
                for c in range(C):
                    ps4 = psum.tile([128, 4, 512], F32, tag="ps", name="ps4")
                    for k in range(KC):
                        for i in range(4):
                            nc.tensor.matmul(ps4[:, i, 0:NPATCH],
                                             lhsT=cT[k][:, c * 128:(c + 1) * 128],
                                             rhs=rhs[pt * 4 + i][:, k, :],
                                             start=(k == 0), stop=(k == KC - 1))
                    nc.vector.reduce_max(out=maxcol[:, c, pt * 4:pt * 4 + 4],
                                         in_=ps4[:, :, 0:NPATCH], axis=AX.X)
                    if c in preps:
                        prep_image(preps[c])

            main_pt(0, preps={2: 8, 6: 9, 10: 10, 14: 11})
            main_pt(1, preps={2: 12, 6: 13, 10: 14, 14: 15})

            main_pt(2)

            # concept row norms (normalization itself is deferred into GT)
            for c in range(C):
                cn = work.tile([128, D], BF, tag="cnat", bufs=3)
                nc.sync.dma_start(out=cn[:], in_=d_cnat.ap()[c * 128:(c + 1) * 128, :])
                rownorm_recip(cn[:], 128, rnorm[:, c:c + 1])

            # GT rows scaled by 1/||c||  (G_eff[v,p] = G[v,p] * rnorm[p])
            GT = consts.tile([128, C * B], F32, tag="GT")
            for c in range(C):
                nc.sync.dma_start(out=GT[:, c * B:(c + 1) * B], in_=d_GT.ap()[c])
                nc.vector.tensor_scalar_mul(GT[:, c * B:(c + 1) * B],
                                            GT[:, c * B:(c + 1) * B],
                                            rnorm[:, c:c + 1])

            # text / image CLS features -> transposed normalized operands
            txt_t = work.tile([128, D], BF, tag="nat", bufs=12)
            nc.sync.dma_start(out=txt_t[:], in_=d_txt.ap())
            norm_transpose(txt_t, 128, txtT, 0, nc.vector.tensor_copy)
            img_t = work.tile([128, D], BF, tag="nat", bufs=12)
            nc.sync.dma_start(out=img_t[0:M_PER], in_=d_img.ap())
            norm_transpose(img_t, M_PER, imgT, 0, nc.scalar.copy)

            # IT-align logits (v, m_local); affine applied at stash time
            itps = psum.tile([128, 4, 512], F32, tag="ps")
            for k in range(KC):
                nc.tensor.matmul(itps[:, 0, 0:M_PER], lhsT=txtT[:, k, :],
                                 rhs=imgT[:, k, :], start=(k == 0),
                                 stop=(k == KC - 1))
            nc.scalar.activation(out=yit[:], in_=itps[:, 0, 0:M_PER], func=AF.Copy,
                                 bias=float(bias), scale=float(t))

            nc.scalar.activation(out=warm[:], in_=warm[:], func=AF.Exp)

            main_pt(3)

            # S[v, m] = sum_p G_eff[v, p] * maxcol[p, m]  (fp32)
            sps = psum.tile([128, 4, 512], F32, tag="ps")
            for c in range(C):
                nc.tensor.matmul(sps[:, 0, 0:M_PER], lhsT=GT[:, c * B:(c + 1) * B],
                                 rhs=maxcol[:, c, :], start=(c == 0),
                                 stop=(c == C - 1))

            # loss elements: softplus(-z*(t*S+bias)) = ln(1 + exp(-z*(t*S+bias)))
            def softplus_out(y_ap, d_out):
                el = small.tile([B, M_PER], F32, tag="el", name="el")
                nc.scalar.activation(out=el[:], in_=y_ap, func=AF.Exp)
                nc.vector.tensor_scalar_add(el[:], el[:], 1.0)
                nc.scalar.activation(out=el[:], in_=el[:], func=AF.Ln)
                nc.sync.dma_start(out=d_out.ap(), in_=el[:])

            yrc = small.tile([B, M_PER], F32, tag="y")
            nc.scalar.activation(out=yrc[:], in_=sps[:, 0, 0:M_PER], func=AF.Copy,
                                 bias=float(bias), scale=float(t))
            nc.vector.tensor_mul(yrc[:], yrc[:], sign[:])
            softplus_out(yrc[:], d_rc)

            nc.vector.tensor_mul(yit[:], yit[:], sign[:])
            softplus_out(yit[:], d_it)

    nc.compile()
    return nc


def _install_trace_hook():
    """Register the axon NTFF profiling hook (missing from this image) so
    run_bass_kernel_spmd(trace=True) can capture HW exec time."""
    import contextlib
    import ctypes
    import types

    import concourse.bass_utils as bu

    if "antenv.axon_hooks" in sys.modules:
        return
    so_path = "/opt/axon/libaxon_pjrt.so"

    def _make_hook():
        lib = ctypes.CDLL(so_path)
        if not hasattr(lib, "axon_start_nrt_profile"):
            return None
        lib.axon_start_nrt_profile.argtypes = [ctypes.POINTER(ctypes.c_int64),
                                               ctypes.c_size_t]
        lib.axon_start_nrt_profile.restype = ctypes.c_int64
        lib.axon_stop_nrt_profile.argtypes = [ctypes.c_char_p]
        lib.axon_stop_nrt_profile.restype = ctypes.c_int64

        @contextlib.contextmanager
        def _hook(output_dir, device_ids):
            import jax
            jax.devices()
            if device_ids:
                ids = (ctypes.c_int64 * len(device_ids))(*device_ids)
                rc = lib.axon_start_nrt_profile(ids, len(device_ids))
            else:
                rc = lib.axon_start_nrt_profile(None, 0)
            if rc != 0:
                raise RuntimeError(f"axon_start_nrt_profile rc={rc}")
            try:
                yield
            finally:
                n = lib.axon_stop_nrt_profile(str(output_dir).encode())
                print(f"profile: {n} file(s) written to {output_dir}",
                      file=sys.stderr)

        return _hook

    mod = types.ModuleType("antenv.axon_hooks")
    mod.get_axon_ntff_profile_hook = _make_hook
    sys.modules["antenv.axon_hooks"] = mod
    bu.upload_artifacts = lambda tmpdir: tmpdir  # no S3 in this container


def _prepare(inputs):
    image_features = np.asarray(inputs["image_features"], np.float32)
    text_features = np.asarray(inputs["text_features"], np.float32)
    image_token_features = np.asarray(inputs["image_token_features"], np.float32)
    concept_text_features = np.asarray(inputs["concept_text_features"], np.float32)
    counts = np.asarray(inputs["concept_counts"]).astype(np.int64)
    t = float(np.exp(np.clip(np.float32(inputs["logit_scale"]), -10.0, 10.0)))
    bias = float(np.float32(inputs["logit_bias"]))

    # pack concepts: keep only w < counts[v]; pad rows with ones (zero weight)
    vidx = np.repeat(np.arange(B), counts)
    widx = np.concatenate([np.arange(c) for c in counts])
    P = len(vidx)
    C = math.ceil(P / 128)
    Ppad = C * 128
    cnat = np.ones((Ppad, D), np.float32)
    cnat[:P] = concept_text_features[vidx, widx]
    cnat_bf = cnat.astype(BF16)
    cT = np.ascontiguousarray(cnat_bf.T).reshape(KC, 128, Ppad)

    G = np.zeros((Ppad, B), np.float32)
    G[np.arange(P), vidx] = 1.0 / counts[vidx]
    GT = G.reshape(C, 128, B)

    txt_bf = text_features.astype(BF16)
    ident = np.eye(128, dtype=BF16)

    in_maps = []
    for core in range(N_CORES):
        s = slice(core * M_PER, (core + 1) * M_PER)
        signneg = np.ones((B, M_PER), np.float32)
        for j in range(M_PER):
            signneg[core * M_PER + j, j] = -1.0
        in_maps.append({
            "patches": image_token_features[s].astype(BF16),
            "cT": cT,
            "cnat": cnat_bf,
            "GT": GT,
            "img": image_features[s].astype(BF16),
            "txt": txt_bf,
            "signneg": signneg,
            "ident": ident,
        })
    return in_maps, C, t, bias


def _run(inputs, trace=False, tmpdir=None):
    in_maps, C, t, bias = _prepare(inputs)
    key = (C, t, bias)
    if key not in _cache:
        _cache[key] = _build(C, t, bias)
    nc = _cache[key]
    kwargs = {}
    if trace:
        _install_trace_hook()
        kwargs = dict(trace=True, tmpdir=tmpdir)
    res = run_bass_kernel_spmd(nc, in_maps, core_ids=list(range(N_CORES)),
                               **kwargs)
    it_sum = sum(float(r["it_el"].astype(np.float64).sum()) for r in res.results)
    rc_sum = sum(float(r["rc_el"].astype(np.float64).sum()) for r in res.results)
    it_loss = it_sum / (B * B)
    rc_loss = rc_sum / (B * B)
    total = it_loss + 0.5 * rc_loss
    out = (np.float32(total), np.float32(it_loss), np.float32(rc_loss))
    return out, res


def kernel(**inputs):
    out, _ = _run(inputs)
    return out



# revision 12
# speedup vs baseline: 1.3077x; 1.3077x over previous
"""ConceptCLIP loss kernel for 8x Trainium2 NeuronCores (Bass/Tile).

Strategy (data-parallel over the image batch axis m):
  - Each core owns 16 of the 128 images; concept/text features are replicated.
  - Concepts are host-packed (only w < counts[v] kept) and quantized to
    fp8e4m3 RAW; their 1/||c|| is folded into the G gather matrix on device.
  - Patches are host-TRANSPOSED to (d, n) layout (pure layout, free) and
    shipped bf16; the per-patch 16/||x|| scale is computed on device from a
    natural-layout bf16 copy (ACT square+accum), turned into per-column
    broadcast tiles (PE transpose of the tiny ssq matrix + GPSIMD
    partition_broadcast), and applied by DVE with fp8e4m3 output.
  - Main loop: fp8 DoubleRow matmuls (contraction 256/instr, 2x ALU rate):
    A[p, n] accumulated over 3 k-pairs into PSUM, 2 images (416 cols) per
    bank, ring of 6 banks; DVE reduce_max drains each bank to maxcol (bf16).
  - S = (G*rnorm/16)^T @ maxcol in bf16; IT-align logits via raw bf16 matmul
    with deferred rank-1 normalization (t/||txt_v|| per-partition scale,
    1/||img_m|| per-column broadcast). Softplus loss elements are DMA'd out;
    host sums them.
"""

import math
import os
import sys

for _p in ("/opt/trn_rl_repo", "/root/.axon_site/_ro/trn_rl_repo"):
    if os.path.isdir(_p) and _p not in sys.path:
        sys.path.insert(0, _p)

import ml_dtypes
import numpy as np

import concourse.tile as tile
from concourse import bacc, mybir
from concourse.bass_utils import run_bass_kernel_spmd

BF16 = ml_dtypes.bfloat16
FP8 = ml_dtypes.float8_e4m3

N_CORES = 8
B, NPATCH, D, W = 128, 196, 768, 32
M_PER = B // N_CORES   # 16 images per core
PAIRS = M_PER // 2     # 8 image pairs
KC = D // 128          # 6 contraction chunks of 128
NKP = KC // 2          # 3 DoubleRow k-pairs (contraction 256 each)
SLOT = 208             # per-image column slot (196 + 12 zero pad, 16B align)
COLS = 2 * SLOT        # 416 columns per pair

F32 = mybir.dt.float32
BF = mybir.dt.bfloat16
F8 = mybir.dt.float8e4
AX = mybir.AxisListType
AF = mybir.ActivationFunctionType
DR = mybir.MatmulPerfMode.DoubleRow

_cache = {}


def _build(C, t, bias):
    """Build + compile the per-core Bass program. C = number of 128-row packed
    concept chunks; t/bias are compile-time scalar constants."""
    P2 = C * 128
    nc = bacc.Bacc("TRN2", target_bir_lowering=False, debug=False,
                   num_devices=N_CORES)

    d_pT = nc.dram_tensor("pT", (PAIRS, 128, KC, COLS), BF, kind="ExternalInput")
    d_cT = nc.dram_tensor("cT", (NKP, 128, 2, P2), F8, kind="ExternalInput")
    d_cnat = nc.dram_tensor("cnat", (P2, D), BF, kind="ExternalInput")
    d_GT = nc.dram_tensor("GT", (C, 128, B), BF, kind="ExternalInput")
    d_txtT = nc.dram_tensor("txtT", (128, KC, 128), BF, kind="ExternalInput")
    d_imgT = nc.dram_tensor("imgT", (128, KC, M_PER), BF, kind="ExternalInput")
    d_txtn = nc.dram_tensor("txtn", (B, D), BF, kind="ExternalInput")
    d_sign = nc.dram_tensor("signneg", (B, M_PER), F32, kind="ExternalInput")
    d_rc = nc.dram_tensor("rc_el", (B, M_PER), F32, kind="ExternalOutput")
    d_it = nc.dram_tensor("it_el", (B, M_PER), F32, kind="ExternalOutput")

    with tile.TileContext(nc) as tc:
        with (
            tc.tile_pool(name="consts", bufs=1) as consts,
            tc.tile_pool(name="work", bufs=3) as work,
            tc.tile_pool(name="small", bufs=4) as small,
            tc.tile_pool(name="psum", bufs=6, space="PSUM") as psum,
        ):
            sign = consts.tile([B, M_PER], F32, tag="sign")
            nc.sync.dma_start(out=sign[:], in_=d_sign.ap())
            ones_col = consts.tile([128, 1], BF, tag="ones")
            nc.vector.memset(ones_col[:], 1.0)
            warm = small.tile([1, 1], F32, tag="warm")
            nc.vector.memset(warm[:], 1.0)
            nc.scalar.activation(out=warm[:], in_=warm[:], func=AF.Square)

            maxcol = consts.tile([128, C, M_PER], BF, tag="maxcol")
            rnorm = consts.tile([128, C], F32, tag="rnorm")
            GTbf = consts.tile([128, C * B], BF, tag="GT")
            yit = consts.tile([B, M_PER], F32, tag="yit")

            pT = [consts.tile([128, KC, COLS], BF, tag=f"pT{p}", name=f"pT{p}")
                  for p in range(PAIRS)]
            rhs8 = [consts.tile([128, KC, COLS], F8, tag=f"r8{p}", name=f"r8{p}")
                    for p in range(PAIRS)]
            cT = []
            for j in range(NKP):
                tj = consts.tile([128, 2, P2], F8, tag=f"cT{j}", name=f"cT{j}")
                nc.sync.dma_start(out=tj[:], in_=d_cT.ap()[j])
                cT.append(tj)

            def prep_pair(pr):
                # per-column (||x_n||/16)^2 via ones-matmul over squared pT
                nc.sync.dma_start(out=pT[pr][:], in_=d_pT.ap()[pr])
                sq = work.tile([128, KC, COLS], BF, tag="sq", bufs=2)
                for k in range(KC):
                    nc.scalar.activation(out=sq[:, k, :], in_=pT[pr][:, k, :],
                                         func=AF.Square, scale=1.0 / 16.0)
                nps = psum.tile([128, 512], F32, tag="aux", bufs=2, name="nps")
                for k in range(KC):
                    nc.tensor.matmul(nps[0:1, 0:COLS], lhsT=ones_col[:, :],
                                     rhs=sq[:, k, :], start=(k == 0),
                                     stop=(k == KC - 1))
                rrow = small.tile([1, COLS], F32, tag="rrow", bufs=2)
                # ssq + 1e-12: zero-pad columns -> 1e-6 after sqrt, recip finite
                nc.vector.tensor_scalar_add(rrow[:], nps[0:1, 0:COLS], 1e-12)
                nc.scalar.sqrt(rrow[:], rrow[:])
                rbrow = small.tile([1, COLS], BF, tag="rbrow", bufs=2)
                with nc.allow_low_precision(reason="scale feeds fp8 quantization"):
                    nc.vector.reciprocal(rbrow[:], rrow[:])   # 16/||x||, bf16
                bc = work.tile([128, COLS], BF, tag="bc", bufs=3)
                nc.gpsimd.partition_broadcast(out_ap=bc[:, :], in_ap=rbrow[0:1, :])
                for k in range(KC):
                    nc.vector.tensor_mul(rhs8[pr][:, k, :], pT[pr][:, k, :],
                                         bc[:, :])

            for pr in range(4):
                prep_pair(pr)

            def main_phase(half, preps=()):
                # A[p_chunk, img cols] via fp8 DoubleRow, 2 images per PSUM
                # bank, ring of 6 banks; reduce_max drains to maxcol.
                preps = dict(preps)
                prs = list(range(half * 4, half * 4 + 4))
                for c in range(C):
                    pss = [psum.tile([128, 512], F32, tag="mm", bufs=6,
                                     name="mm") for _ in prs]
                    for j in range(NKP):
                        for i, pr in enumerate(prs):
                            nc.tensor.matmul(pss[i][:, 0:COLS],
                                             lhsT=cT[j][:, :, c * 128:(c + 1) * 128],
                                             rhs=rhs8[pr][:, 2 * j:2 * j + 2, :],
                                             start=(j == 0), stop=(j == NKP - 1),
                                             perf_mode=DR)
                    for i, pr in enumerate(prs):
                        nc.vector.reduce_max(
                            out=maxcol[:, c, 2 * pr:2 * pr + 2],
                            in_=pss[i][:, 0:COLS].rearrange("p (s x) -> p s x", s=2),
                            axis=AX.X)
                    if c in preps:
                        prep_pair(preps[c])

            main_phase(0, preps={1: 4, 4: 5, 7: 6, 10: 7})

            # ---- concept row norms + G scaling (overlaps with main loop) ----
            for c in range(C):
                cn = work.tile([128, D], BF, tag="cnat", bufs=3)
                nc.sync.dma_start(out=cn[:], in_=d_cnat.ap()[c * 128:(c + 1) * 128, :])
                scr = work.tile([128, D], BF, tag="scr", bufs=3)
                cs = small.tile([128, 1], F32, tag="css", bufs=8)
                nc.scalar.activation(out=scr[:], in_=cn[:], func=AF.Square,
                                     accum_out=cs[:])
                nc.scalar.sqrt(cs[:], cs[:])
                nc.vector.reciprocal(rnorm[:, c:c + 1], cs[:])
            for c in range(C):
                nc.sync.dma_start(out=GTbf[:, c * B:(c + 1) * B], in_=d_GT.ap()[c])
                nc.vector.tensor_scalar_mul(GTbf[:, c * B:(c + 1) * B],
                                            GTbf[:, c * B:(c + 1) * B],
                                            rnorm[:, c:c + 1])

            # ---- IT-align: raw bf16 matmul + deferred rank-1 normalization --
            txtT = consts.tile([128, KC, 128], BF, tag="txtT")
            nc.sync.dma_start(out=txtT[:], in_=d_txtT.ap())
            imgT = consts.tile([128, KC, M_PER], BF, tag="imgT")
            nc.sync.dma_start(out=imgT[:], in_=d_imgT.ap())
            txtn = work.tile([128, D], BF, tag="nat", bufs=10)
            nc.sync.dma_start(out=txtn[:], in_=d_txtn.ap())
            tscr = work.tile([128, D], BF, tag="scr", bufs=3)
            tss = small.tile([128, 1], F32, tag="tss")
            nc.scalar.activation(out=tscr[:], in_=txtn[:], func=AF.Square,
                                 accum_out=tss[:])
            nc.scalar.sqrt(tss[:], tss[:])
            av = small.tile([128, 1], F32, tag="av")
            nc.vector.reciprocal(av[:], tss[:])
            nc.vector.tensor_scalar_mul(av[:], av[:], float(t))  # t/||txt_v||

            # img norms via ones-matmul on squared imgT (transposed layout)
            isq = small.tile([128, KC, M_PER], BF, tag="isq")
            nc.scalar.activation(out=isq[:], in_=imgT[:], func=AF.Square)
            ips = psum.tile([128, 512], F32, tag="aux", bufs=2, name="ips")
            for k in range(KC):
                nc.tensor.matmul(ips[0:1, 0:M_PER], lhsT=ones_col[:, :],
                                 rhs=isq[:, k, :], start=(k == 0),
                                 stop=(k == KC - 1))
            ib = small.tile([1, M_PER], F32, tag="ib")
            nc.scalar.sqrt(ib[:], ips[0:1, 0:M_PER])
            nc.vector.reciprocal(ib[:], ib[:])                   # 1/||img_m||
            bg = small.tile([128, M_PER], F32, tag="bg")
            nc.gpsimd.partition_broadcast(out_ap=bg[:, :], in_ap=ib[0:1, :])

            itps = psum.tile([128, 512], F32, tag="aux", bufs=2, name="itps")
            for k in range(KC):
                nc.tensor.matmul(itps[:, 0:M_PER], lhsT=txtT[:, k, :],
                                 rhs=imgT[:, k, :], start=(k == 0),
                                 stop=(k == KC - 1))
            nc.scalar.activation(out=yit[:], in_=itps[:, 0:M_PER], func=AF.Copy,
                                 scale=av[:])
            nc.vector.tensor_mul(yit[:], yit[:], bg[:])
            nc.vector.tensor_scalar_add(yit[:], yit[:], float(bias))

            nc.scalar.activation(out=warm[:], in_=warm[:], func=AF.Exp)

            main_phase(1)

            # ---- S[v, m] = sum_p Geff[p,v] * maxcol[p,m]  (bf16, fp32 acc) --
            sps = psum.tile([128, 512], F32, tag="aux", bufs=2, name="sps")
            for c in range(C):
                nc.tensor.matmul(sps[:, 0:M_PER], lhsT=GTbf[:, c * B:(c + 1) * B],
                                 rhs=maxcol[:, c, :], start=(c == 0),
                                 stop=(c == C - 1))

            def softplus_out(y_ap, d_out):
                el = small.tile([B, M_PER], F32, tag="el", name="el")
                nc.scalar.activation(out=el[:], in_=y_ap, func=AF.Exp)
                nc.vector.tensor_scalar_add(el[:], el[:], 1.0)
                nc.scalar.activation(out=el[:], in_=el[:], func=AF.Ln)
                nc.sync.dma_start(out=d_out.ap(), in_=el[:])

            yrc = small.tile([B, M_PER], F32, tag="y")
            nc.scalar.activation(out=yrc[:], in_=sps[:, 0:M_PER], func=AF.Copy,
                                 bias=float(bias), scale=float(t))
            nc.vector.tensor_mul(yrc[:], yrc[:], sign[:])
            softplus_out(yrc[:], d_rc)

            nc.vector.tensor_mul(yit[:], yit[:], sign[:])
            softplus_out(yit[:], d_it)

    nc.compile()
    return nc


def _install_trace_hook():
    """Register the axon NTFF profiling hook (missing from this image) so
    run_bass_kernel_spmd(trace=True) can capture HW exec time."""
    import contextlib
    import ctypes
    import types

    import concourse.bass_utils as bu

    if "antenv.axon_hooks" in sys.modules:
        return
    so_path = "/opt/axon/libaxon_pjrt.so"

    def _make_hook():
        lib = ctypes.CDLL(so_path)
        if not hasattr(lib, "axon_start_nrt_profile"):
            return None
        lib.axon_start_nrt_profile.argtypes = [ctypes.POINTER(ctypes.c_int64),
                                               ctypes.c_size_t]
        lib.axon_start_nrt_profile.restype = ctypes.c_int64
        lib.axon_stop_nrt_profile.argtypes = [ctypes.c_char_p]
        lib.axon_stop_nrt_profile.restype = ctypes.c_int64

        @contextlib.contextmanager
        def _hook(output_dir, device_ids):
            import jax
            jax.devices()
            if device_ids:
                ids = (ctypes.c_int64 * len(device_ids))(*device_ids)
                rc = lib.axon_start_nrt_profile(ids, len(device_ids))
            else:
                rc = lib.axon_start_nrt_profile(None, 0)
            if rc != 0:
                raise RuntimeError(f"axon_start_nrt_profile rc={rc}")
            try:
                yield
            finally:
                n = lib.axon_stop_nrt_profile(str(output_dir).encode())
                print(f"profile: {n} file(s) written to {output_dir}",
                      file=sys.stderr)

        return _hook

    mod = types.ModuleType("antenv.axon_hooks")
    mod.get_axon_ntff_profile_hook = _make_hook
    sys.modules["antenv.axon_hooks"] = mod
    bu.upload_artifacts = lambda tmpdir: tmpdir  # no S3 in this container


def _prepare(inputs):
    image_features = np.asarray(inputs["image_features"], np.float32)
    text_features = np.asarray(inputs["text_features"], np.float32)
    image_token_features = np.asarray(inputs["image_token_features"], np.float32)
    concept_text_features = np.asarray(inputs["concept_text_features"], np.float32)
    counts = np.asarray(inputs["concept_counts"]).astype(np.int64)
    t = float(np.exp(np.clip(np.float32(inputs["logit_scale"]), -10.0, 10.0)))
    bias = float(np.float32(inputs["logit_bias"]))

    # pack concepts: keep only w < counts[v]; pad rows with ones (zero weight)
    vidx = np.repeat(np.arange(B), counts)
    widx = np.concatenate([np.arange(c) for c in counts])
    P = len(vidx)
    C = math.ceil(P / 128)
    P2 = C * 128
    cnat = np.ones((P2, D), np.float32)
    cnat[:P] = concept_text_features[vidx, widx]
    cnat_bf = cnat.astype(BF16)
    # cT[j, d128, h, p] = fp8(cnat[p, (2j+h)*128 + d])
    cT = np.ascontiguousarray(
        cnat.astype(FP8).T.reshape(NKP, 2, 128, P2).transpose(0, 2, 1, 3))

    # G with 1/(16*counts): folds away the x16 patch scale
    G = np.zeros((P2, B), np.float32)
    G[np.arange(P), vidx] = 1.0 / (16.0 * counts[vidx])
    GT = G.astype(BF16).reshape(C, 128, B)

    txt_bf = text_features.astype(BF16)
    # txtT[d, k, v] = txt_bf[v, k*128 + d]
    txtT = np.ascontiguousarray(
        txt_bf.T.reshape(KC, 128, B).transpose(1, 0, 2))

    in_maps = []
    for core in range(N_CORES):
        s = slice(core * M_PER, (core + 1) * M_PER)
        sh = image_token_features[s].astype(BF16)        # (16, 196, 768)
        pT = np.zeros((PAIRS, 128, KC, COLS), BF16)
        shT = sh.transpose(0, 2, 1).reshape(M_PER, KC, 128, NPATCH)
        # pT[pr, d, k, slot*i2 + n] = patches[2pr+i2][n, k*128+d]
        pT[:, :, :, 0:NPATCH] = shT[0::2].transpose(0, 2, 1, 3)
        pT[:, :, :, SLOT:SLOT + NPATCH] = shT[1::2].transpose(0, 2, 1, 3)
        img_bf = image_features[s].astype(BF16)          # (16, 768)
        imgT = np.ascontiguousarray(
            img_bf.T.reshape(KC, 128, M_PER).transpose(1, 0, 2))
        signneg = np.ones((B, M_PER), np.float32)
        for j in range(M_PER):
            signneg[core * M_PER + j, j] = -1.0
        in_maps.append({
            "pT": pT,
            "cT": cT,
            "cnat": cnat_bf,
            "GT": GT,
            "txtT": txtT,
            "imgT": imgT,
            "txtn": txt_bf,
            "signneg": signneg,
        })
    return in_maps, C, t, bias


def _run(inputs, trace=False, tmpdir=None):
    in_maps, C, t, bias = _prepare(inputs)
    key = (C, t, bias)
    if key not in _cache:
        _cache[key] = _build(C, t, bias)
    nc = _cache[key]
    kwargs = {}
    if trace:
        _install_trace_hook()
        kwargs = dict(trace=True, tmpdir=tmpdir)
    res = run_bass_kernel_spmd(nc, in_maps, core_ids=list(range(N_CORES)),
                               **kwargs)
    it_sum = sum(float(r["it_el"].astype(np.float64).sum()) for r in res.results)
    rc_sum = sum(float(r["rc_el"].astype(np.float64).sum()) for r in res.results)
    it_loss = it_sum / (B * B)
    rc_loss = rc_sum / (B * B)
    total = it_loss + 0.5 * rc_loss
    out = (np.float32(total), np.float32(it_loss), np.float32(rc_loss))
    return out, res


def kernel(**inputs):
    out, _ = _run(inputs)
    return out
